# revision 3
# baseline (speedup 1.0000x reference)
"""Trainium2 Bass kernel for nn_CreateOverlappingWindows.

out[b, t, w*C + c] = x_padded[b, t + w, c]  (SAME zero padding, n_context=9)

Key identity: flattening (w, c) -> 494 contiguous values, each output row is
a contiguous 494-element window of the zero-padded flattened input:
    out[b, t, :] = xpad_flat[b, t*C : t*C + W*C]

v2: stage the (tiny) padded input in SBUF once, then emit the 19x-expanded
output with overlapping-window gather DMAs reading SBUF instead of HBM.
This halves HBM traffic vs the DRAM->DRAM baseline (15.8MB read saved/core).

SBUF layout (per core): one block of time rows + the 18-row window halo per
partition, so every output row's 494-elem window lives within one partition:
  partition 16*b + k       (k in [0,16)): 63 rows, t0 = 63*k   (rows 0..1007)
  partition 64 + 16*b + k  (k in [0,16)): 62 rows, t0 = 1008+62*k
All 128 partitions are used and carry ~equal bytes, so all 16 SBUF AXI
ports are balanced and the output is HBM-write-bound (~358 GB/s/core).

Sharding: pure data parallel - batch 32 split 4-per-core across 8 cores.
Host zero-pads x (936 bytes per row-edge) so no edge cases on device.
"""

import sys

sys.path.insert(0, "/opt/trn_rl_repo")

import numpy as np
from concourse import bass, mybir
from concourse.ap import AP
from concourse.bass_utils import run_bass_kernel_spmd

_F32 = mybir.dt.float32

_NCORES = 8
_B, _T, _C = 32, 2000, 26
_NCTX = 9
_W = 2 * _NCTX + 1  # 19
_WC = _W * _C  # 494
_PAD = _NCTX * _C  # 234
_BPC = _B // _NCORES  # 4 batches per core
_NP = _T * _C + 2 * _PAD  # 52468 padded flat length per batch
_TWC = _T * _WC

# SBUF blocking: per batch, 16 "A" blocks of 63 rows + 16 "B" blocks of 62.
_RA, _RB = 63, 62
_NB = 16  # blocks per geometry per batch
_TA = _RA * _NB  # 1008 rows covered by A
_HALO = (_W - 1) * _C  # 468
_LA = _RA * _C + _HALO  # 2106 f32 per A partition
_LB = _RB * _C + _HALO  # 2080 f32 per B partition
_SBF = _LA  # sbuf free-dim length (covers both)

_nc_cache = None


def _build():
    global _nc_cache
    if _nc_cache is not None:
        return _nc_cache
    nc = bass.Bass()
    xp = nc.declare_dram_parameter("xp", [_BPC, _NP], _F32, isOutput=False)
    out = nc.declare_dram_parameter("out", [_BPC, _T, _WC], _F32, isOutput=True)

    with (
        nc.sbuf_tensor([128, _SBF], _F32) as sb,
        nc.Block() as block,
        nc.semaphore("sA") as sA,
        nc.semaphore("sB") as sB,
    ):
        sbt = sb.tensor if hasattr(sb, "tensor") else sb

        @block.sync
        def _(e):
            # load all A blocks: partitions 0..63 (p = 16*b + k), halo-overlapped
            e.dma_start(
                out=AP(sbt, 0, [[_SBF, 4 * _NB], [1, _LA]]),
                in_=AP(xp, 0, [[_NP, _BPC], [_RA * _C, _NB], [1, _LA]]),
            ).then_inc(sA, 16)
            # load all B blocks: partitions 64..127
            e.dma_start(
                out=AP(sbt, 64 * _SBF, [[_SBF, 4 * _NB], [1, _LB]]),
                in_=AP(xp, _TA * _C, [[_NP, _BPC], [_RB * _C, _NB], [1, _LB]]),
            ).then_inc(sB, 16)
            e.wait_ge(sA, 16)
            for b in range(_BPC):
                e.dma_start(
                    out=AP(out, b * _TWC, [[1, _NB * _RA * _WC]]),
                    in_=AP(
                        sbt,
                        16 * b * _SBF,
                        [[_SBF, _NB], [_C, _RA], [1, _WC]],
                    ),
                ).then_inc(sA, 16)
            e.wait_ge(sA, 16 + 16 * _BPC)

        @block.scalar
        def _(e):
            e.wait_ge(sB, 16)
            for b in range(_BPC):
                e.dma_start(
                    out=AP(
                        out,
                        b * _TWC + _TA * _WC,
                        [[1, _NB * _RB * _WC]],
                    ),
                    in_=AP(
                        sbt,
                        (64 + 16 * b) * _SBF,
                        [[_SBF, _NB], [_C, _RB], [1, _WC]],
                    ),
                ).then_inc(sB, 16)
            e.wait_ge(sB, 16 + 16 * _BPC)

    _nc_cache = nc
    return nc


def kernel(x: np.ndarray) -> np.ndarray:
    x = np.asarray(x, dtype=np.float32)  # tolerate jax arrays / views
    assert x.shape == (_B, _T, _C), x.shape
    nc = _build()

    xp = np.zeros((_B, _NP), np.float32)
    xp[:, _PAD : _PAD + _T * _C] = x.reshape(_B, _T * _C)

    in_maps = [
        {"xp": np.ascontiguousarray(xp[i * _BPC : (i + 1) * _BPC])}
        for i in range(_NCORES)
    ]
    res = run_bass_kernel_spmd(nc, in_maps, list(range(_NCORES)))
    return np.concatenate([r["out"] for r in res.results], axis=0)


# revision 7
# speedup vs baseline: 1.4652x; 1.4652x over previous
"""Trainium2 Bass kernel for nn_CreateOverlappingWindows.

out[b, t, w*C + c] = x_padded[b, t + w, c]  (SAME zero padding, n_context=9)

Key identity: flattening (w, c) -> 494 contiguous values, each output row is
a contiguous 494-element window of the zero-padded flattened input:
    out[b, t, :] = xpad_flat[b, t*C : t*C + W*C]

v4: overlapping-window gather DMAs are descriptor-bound (~100 GB/s/queue
measured), so instead: stage input in SBUF, expand the 19 shifted copies with
DVE (contiguous source runs), and stream the expanded output with large
contiguous per-partition descriptors. DMA engines stripe over the OUTER AP
dim (measured: outer dim 4 -> only 4 of 16 engines), so output DMAs put the
32 partition-blocks in the outer dim and the batch pair in the middle dim,
keeping all 16 engines/SBUF ports busy. Two queues (sync=batches 0/1,
scalar=batches 2/3) run concurrently; a third (gpsimd) handles the tail.

Layout (per core, 4 batches): uniform blocks of 62 rows. Partition
p = 64h + 2j + g holds batch 2h+g, rows [62j, 62j+62) plus the 18-row
window halo (input, 2080 f32) and the expanded output (62*494 f32).
(SBUF DMA APs allow only one partition dim, so the SBUF side is a single
64-partition dim; the DRAM side enumerates (block outer, batch-lsb inner)
to match, and its 32-count outer dim stripes across all 16 DMA engines.)
Rows 1984..1999 of each batch ("tail") live on partitions 0..3 in a
spare free-dim region, expanded the same way, streamed on the gpsimd queue.
The expansion pipelines over 4 row-chunks so DVE overlaps the streaming.

Sharding: pure data parallel - batch 32 split 4-per-core across 8 cores.
Host zero-pads x (936 bytes per row-edge) so no edge cases on device.
"""

import sys

sys.path.insert(0, "/opt/trn_rl_repo")

import numpy as np
from concourse import bass, mybir
from concourse.ap import AP
from concourse.bass_utils import run_bass_kernel_spmd

_F32 = mybir.dt.float32

_NCORES = 8
_B, _T, _C = 32, 2000, 26
_NCTX = 9
_W = 2 * _NCTX + 1  # 19
_WC = _W * _C  # 494
_PAD = _NCTX * _C  # 234
_BPC = _B // _NCORES  # 4 batches per core
_NP = _T * _C + 2 * _PAD  # 52468 padded flat length per batch
_TWC = _T * _WC

_R = 62  # rows per block
_NBLK = 32  # blocks per batch
_TM = _R * _NBLK  # 1984 rows covered by main blocks
_NT = _T - _TM  # 16 tail rows per batch
_HALO = (_W - 1) * _C  # 468
_LIN = _R * _C + _HALO  # 2080 f32 main input per partition
_LTIN = _NT * _C + _HALO  # 884 f32 tail input
_INF = _LIN + _LTIN  # 2964 in_sb free length
_LOUT = _R * _WC  # 30628 main output per partition
_LTOUT = _NT * _WC  # 7904 tail output
_OUTF = _LOUT + _LTOUT  # 38532 out_sb free length

_CHUNKS = [(0, 8), (8, 14), (22, 18), (40, 22)]  # (r0, nr) covering 62 rows

_nc_cache = None


def _build():
    global _nc_cache
    if _nc_cache is not None:
        return _nc_cache
    nc = bass.Bass()
    xp = nc.declare_dram_parameter("xp", [_BPC, _NP], _F32, isOutput=False)
    out = nc.declare_dram_parameter("out", [_BPC, _T, _WC], _F32, isOutput=True)

    with (
        nc.sbuf_tensor([128, _INF], _F32) as in_sb,
        nc.sbuf_tensor([128, _OUTF], _F32) as out_sb,
        nc.Block() as block,
        nc.semaphore("sIn") as sIn,
        nc.semaphore("sCp") as sCp,
        nc.semaphore("sTl") as sTl,
        nc.semaphore("sOA") as sOA,
        nc.semaphore("sOB") as sOB,
        nc.semaphore("sTT") as sTT,
    ):
        isbt = in_sb.tensor if hasattr(in_sb, "tensor") else in_sb
        osbt = out_sb.tensor if hasattr(out_sb, "tensor") else out_sb

        def out_dma(e, h, r0, nr):
            return e.dma_start(
                out=AP(
                    out,
                    2 * h * _TWC + r0 * _WC,
                    [[_R * _WC, _NBLK], [_TWC, 2], [1, nr * _WC]],
                ),
                in_=AP(
                    osbt,
                    64 * h * _OUTF + r0 * _WC,
                    [[_OUTF, 2 * _NBLK], [1, nr * _WC]],
                ),
            )

        def in_dma(e, h):
            return e.dma_start(
                out=AP(
                    isbt,
                    64 * h * _INF,
                    [[_INF, 2 * _NBLK], [1, _LIN]],
                ),
                in_=AP(
                    xp,
                    2 * h * _NP,
                    [[_R * _C, _NBLK], [_NP, 2], [1, _LIN]],
                ),
            )

        @block.sync
        def _(e):
            # tail inputs -> partitions 0/32/64/96 at free offset _LIN
            e.dma_start(
                out=AP(isbt, _LIN, [[_INF, _BPC], [1, _LTIN]]),
                in_=AP(xp, _TM * _C, [[_NP, _BPC], [1, _LTIN]]),
            ).then_inc(sIn, 16)
            in_dma(e, 0).then_inc(sIn, 16)
            for c, (r0, nr) in enumerate(_CHUNKS):
                e.wait_ge(sCp, c + 1)
                out_dma(e, 0, r0, nr).then_inc(sOA, 16)
            e.wait_ge(sOA, 16 * len(_CHUNKS))

        @block.scalar
        def _(e):
            in_dma(e, 1).then_inc(sIn, 16)
            for c, (r0, nr) in enumerate(_CHUNKS):
                e.wait_ge(sCp, c + 1)
                out_dma(e, 1, r0, nr).then_inc(sOB, 16)
            e.wait_ge(sOB, 16 * len(_CHUNKS))

        @block.vector
        def _(v):
            v.wait_ge(sIn, 48)
            for c, (r0, nr) in enumerate(_CHUNKS):
                for w in range(_W):
                    i = v.tensor_copy(
                        AP(
                            osbt,
                            r0 * _WC + w * _C,
                            [[_OUTF, 128], [_WC, nr], [1, _C]],
                        ),
                        AP(isbt, (r0 + w) * _C, [[_INF, 128], [1, nr * _C]]),
                    )
                    if w == _W - 1:
                        i.then_inc(sCp, 1)
                if c == 1:
                    # tail expansion (4 partitions; gpsimd streams it out)
                    for w in range(_W):
                        i = v.tensor_copy(
                            AP(
                                osbt,
                                _LOUT + w * _C,
                                [[_OUTF, _BPC], [_WC, _NT], [1, _C]],
                            ),
                            AP(
                                isbt,
                                _LIN + w * _C,
                                [[_INF, _BPC], [1, _NT * _C]],
                            ),
                        )
                        if w == _W - 1:
                            i.then_inc(sTl, 1)

        @block.gpsimd
        def _(e):
            e.wait_ge(sTl, 1)
            e.dma_start(
                out=AP(out, _TM * _WC, [[_TWC, _BPC], [1, _LTOUT]]),
                in_=AP(osbt, _LOUT, [[_OUTF, _BPC], [1, _LTOUT]]),
            ).then_inc(sTT, 16)
            e.wait_ge(sTT, 16)

    _nc_cache = nc
    return nc


def kernel(x: np.ndarray) -> np.ndarray:
    x = np.asarray(x, dtype=np.float32)  # tolerate jax arrays / views
    assert x.shape == (_B, _T, _C), x.shape
    nc = _build()

    xp = np.zeros((_B, _NP), np.float32)
    xp[:, _PAD : _PAD + _T * _C] = x.reshape(_B, _T * _C)

    in_maps = [
        {"xp": np.ascontiguousarray(xp[i * _BPC : (i + 1) * _BPC])}
        for i in range(_NCORES)
    ]
    res = run_bass_kernel_spmd(nc, in_maps, list(range(_NCORES)))
    return np.concatenate([r["out"] for r in res.results], axis=0)
